# revision 16
# baseline (speedup 1.0000x reference)
"""Trainium2 Bass kernel for KANPolyLayer:
    y[b,o] = sum_{i,p} x[b,i]^p * coeffs[o,i,p] + bias[o],  p = 0..4

Math: y = sum_{p=1..4} (x^p) @ C_p^T + biascol, where C_p = coeffs[:, :, p]
and biascol[o] = bias[o] + sum_i coeffs[o,i,0] is folded on the host.

Precision split (rel-err budget 2e-2, measured 7.4e-3 on the real inputs):
  planes p=1,2  -> fp8e4 operands, fused into ONE DoubleRow matmul per
                   (k-tile, o-tile, half): 2 fp8 weights/PE cell, so both
                   planes stream in ~1.13x the time of one bf16 plane.
  planes p=3,4  -> bf16 operands (full PE rate + fast-weight-load).
All coefficients are pre-scaled by 512 on the host so the fp8 values sit
in e4m3's normal range; PSUM accumulates 512*y in fp32 and the evacuation
applies the 1/512 descale together with the bias column.

Per-core schedule: power slabs ([i, b] layout) are built per k-tile from
the fp32 x (ScalarE squares, VectorE mul/copies) and stay resident;
coefficient tiles stream through rings.  All 8 output groups (4 o-tiles x
2 b-halves) accumulate concurrently in 8 PSUM banks for k=0..5, then the
last 2 k-tiles run group-major so groups finish staggered: each group's
descale+bias evacuation and output DMA overlap the remaining matmul
stream.  Input loads issue from the sync-engine DMA queue; output stores
issue from the scalar-engine queue so they never sit behind input traffic.

Sharding (8 cores): 4 batch groups x 2 out-dim groups.
  core c -> (bg, og) = (c // 2, c % 2)
Each core computes a disjoint (512 x 1024) block of yT; host gathers.
"""

from contextlib import ExitStack

import numpy as np
import ml_dtypes

import concourse.bacc as bacc
import concourse.bass as bass
import concourse.mybir as mybir
import concourse.tile as tile
from concourse.bass_utils import run_bass_kernel_spmd

F32 = mybir.dt.float32
BF16 = mybir.dt.bfloat16
F8 = mybir.dt.float8e4

B, I, O = 4096, 1024, 1024  # batch, in_dim, out_dim
BW, OW = 4, 2               # batch groups x out-dim groups (8 cores)
BS, OS = B // BW, O // OW   # per-core batch (1024) and out (512)
NK = I // 128               # contraction tiles (8)
NT = OS // 128              # o-tiles (4)
NH = BS // 512              # b-halves (2)
NTAIL = 2                   # trailing k-planes emitted group-contiguous
NWARM = 3                   # fp32 warmup matmuls (each = LOW+HIGH pass, ~1.27us)
CSCALE = 512.0              # host coefficient scale (fp8 range placement)

_CACHE: dict = {}


def _build():
    nc = bacc.Bacc("TRN2", target_bir_lowering=False, debug=False, num_devices=8)

    xt = nc.dram_tensor("xt", [I, BS], F32, kind="ExternalInput")        # [i, b]
    ct12 = nc.dram_tensor("ct12", [I, 2, OS], F8, kind="ExternalInput")  # fp8 p1,p2
    ct34 = nc.dram_tensor("ct34", [2, I, OS], BF16, kind="ExternalInput")
    biasc = nc.dram_tensor("biasc", [OS, 1], F32, kind="ExternalInput")
    yt = nc.dram_tensor("yt", [OS, BS], F32, kind="ExternalOutput")      # [o, b]

    DR = mybir.MatmulPerfMode.DoubleRow

    with tile.TileContext(nc) as tc, ExitStack() as ctx:
        cons = ctx.enter_context(tc.tile_pool(name="cons", bufs=1))
        c12pool = ctx.enter_context(tc.tile_pool(name="c12", bufs=8))
        c34pool = ctx.enter_context(tc.tile_pool(name="c34", bufs=12))
        xpool = ctx.enter_context(tc.tile_pool(name="xin", bufs=5))
        ppool = ctx.enter_context(tc.tile_pool(name="pow", bufs=1))
        opool = ctx.enter_context(tc.tile_pool(name="out", bufs=1))
        pspool = ctx.enter_context(
            tc.tile_pool(name="ps", bufs=8, space=bass.MemorySpace.PSUM)
        )

        # 8 concurrent accumulation groups: (o-tile, b-half) -> one PSUM bank
        ps = {}
        for ot in range(NT):
            for h in range(NH):
                ps[(ot, h)] = pspool.tile(
                    [128, 512], F32, tag="ps", name=f"ps_{ot}_{h}"
                )

        # PE warmup: fp32 garbage matmuls (1/4 rate -> long busy per inst,
        # no conversion dependency) so the HAM clock-gate reaches 2.4 GHz
        # before the real stream starts.
        wz = cons.tile([128, 512], F32)
        nc.gpsimd.memset(wz[:], 0.0)
        for w in range(NWARM):
            nc.tensor.matmul(
                ps[(0, 0)][:, 0:256], wz[:, 0:128], wz[:, 0:256],
                start=True, stop=True, skip_group_check=True,
            )

        # bias column biascol[o-part, ot]: tiny loads on the gpsimd SWDGE
        # queue (emitted after the warmup memset so they don't delay it; the
        # column is only needed at evacuation time ~40us in).
        biascol = cons.tile([128, NT], F32)
        for ot in range(NT):
            nc.gpsimd.dma_start(
                biascol[:, ot:ot + 1], biasc[ot * 128:(ot + 1) * 128, :]
            )

        pows = {}   # (k, 'dr'|3|4, h) -> power tile (k0 h0: chunk pair)
        cp12 = {}   # k -> fp8 [128, 2, OS] tile
        cp34 = {}   # (k, p) -> bf16 [128, OS] tile
        for k in range(NK):
            tail_k = k >= NK - NTAIL
            # k0 critical path: first loads on the (otherwise idle at the
            # start) scalar HWDGE ring so their completion receipts aren't
            # queued behind the bulk input burst on the sync ring, and the
            # first x tile arrives as two 128KB column chunks so the power
            # pipeline (square -> mul/casts -> matmul) starts ~1us sooner.
            if k == 0:
                c = c34pool.tile([128, OS], BF16, tag="c34", name="cp34_0_3")
                nc.scalar.dma_start(c[:], ct34[0, 0:128, :])
                cp34[(0, 3)] = c
            for h2 in range(NH):
                if k == 0 and h2 == 0:
                    chunks = []
                    for cc in range(2):
                        cs = slice(cc * 256, (cc + 1) * 256)
                        x1c = xpool.tile([128, 256], F32, tag=f"x1c{cc}",
                                         name=f"x1c_{cc}")
                        nc.scalar.dma_start(x1c[:], xt[0:128, cs])
                        p12c = ppool.tile([128, 2, 256], F8, tag=f"p12c{cc}",
                                          name=f"p12c_{cc}")
                        p3c = ppool.tile([128, 256], BF16, tag=f"p3c{cc}",
                                         name=f"p3c_{cc}")
                        p4c = ppool.tile([128, 256], BF16, tag=f"p4c{cc}",
                                         name=f"p4c_{cc}")
                        p2fc = xpool.tile([128, 256], F32, tag=f"p2fc{cc}",
                                          name=f"p2fc_{cc}")
                        nc.scalar.square(p2fc[:], x1c[:])
                        nc.vector.tensor_mul(p3c[:], p2fc[:], x1c[:])
                        nc.vector.tensor_copy(p12c[:, 0, :], x1c[:])
                        nc.vector.tensor_copy(p12c[:, 1, :], p2fc[:])
                        nc.scalar.square(p4c[:], p2fc[:])
                        chunks.append({'dr': p12c, 3: p3c, 4: p4c})
                    pows[(0, 'dr', 0)] = (chunks[0]['dr'], chunks[1]['dr'])
                    pows[(0, 3, 0)] = (chunks[0][3], chunks[1][3])
                    pows[(0, 4, 0)] = (chunks[0][4], chunks[1][4])
                    continue
                x1 = xpool.tile([128, 512], F32, tag="x1", name=f"x1_{k}_{h2}")
                (nc.scalar if k == 0 else nc.sync).dma_start(
                    x1[:],
                    xt[k * 128:(k + 1) * 128, h2 * 512:(h2 + 1) * 512],
                )
                p12q = ppool.tile([128, 2, 512], F8, tag=f"p12_{k}_{h2}",
                                  name=f"p12_{k}_{h2}")
                p3b = ppool.tile([128, 512], BF16, tag=f"p3_{k}_{h2}",
                                 name=f"p3_{k}_{h2}")
                p4b = ppool.tile([128, 512], BF16, tag=f"p4_{k}_{h2}",
                                 name=f"p4_{k}_{h2}")
                p2f = xpool.tile([128, 512], F32, tag="p2f", name=f"p2f_{k}_{h2}")
                nc.scalar.square(p2f[:], x1[:])           # x^2 fp32
                nc.vector.tensor_mul(p3b[:], p2f[:], x1[:])   # x^3 -> bf16
                nc.vector.tensor_copy(p12q[:, 0, :], x1[:])   # x -> fp8
                nc.vector.tensor_copy(p12q[:, 1, :], p2f[:])  # x^2 -> fp8
                nc.scalar.square(p4b[:], p2f[:])              # x^4 -> bf16
                pows[(k, 'dr', h2)] = p12q
                pows[(k, 3, h2)] = p3b
                pows[(k, 4, h2)] = p4b

            c = c12pool.tile([128, 2, OS], F8, tag="c12", name=f"cp12_{k}")
            nc.sync.dma_start(c[:], ct12[k * 128:(k + 1) * 128, :, :])
            cp12[k] = c
            for p in (3, 4):
                if (k, p) in cp34:
                    continue
                c = c34pool.tile([128, OS], BF16, tag="c34", name=f"cp34_{k}_{p}")
                nc.sync.dma_start(c[:], ct34[p - 3, k * 128:(k + 1) * 128, :])
                cp34[(k, p)] = c

            if not tail_k:
                if k == 0:
                    # p3 first (start=True clears each bank; shortest power
                    # chain), h0 as two N=256 column chunks so matmuls start
                    # as soon as the first 128KB of x has landed; DR last so
                    # its longer power chain (2 fp8 casts) is off the
                    # critical path.  A chunk-0 matmul carries start=True
                    # (clears the whole bank); chunk 1 then start=False.
                    for cc in range(2):
                        cs = slice(cc * 256, (cc + 1) * 256)
                        for ot in range(NT):
                            nc.tensor.matmul(
                                ps[(ot, 0)][:, cs],
                                cp34[(0, 3)][:, ot * 128:(ot + 1) * 128],
                                pows[(0, 3, 0)][cc][:],
                                start=(cc == 0),
                                stop=False,
                            )
                    for ot in range(NT):
                        nc.tensor.matmul(
                            ps[(ot, 1)],
                            cp34[(0, 3)][:, ot * 128:(ot + 1) * 128],
                            pows[(0, 3, 1)][:],
                            start=True,
                            stop=False,
                        )
                    for cc in range(2):
                        cs = slice(cc * 256, (cc + 1) * 256)
                        for ot in range(NT):
                            nc.tensor.matmul(
                                ps[(ot, 0)][:, cs],
                                cp34[(0, 4)][:, ot * 128:(ot + 1) * 128],
                                pows[(0, 4, 0)][cc][:],
                                start=False,
                                stop=False,
                            )
                    for ot in range(NT):
                        nc.tensor.matmul(
                            ps[(ot, 1)],
                            cp34[(0, 4)][:, ot * 128:(ot + 1) * 128],
                            pows[(0, 4, 1)][:],
                            start=False,
                            stop=False,
                        )
                    for cc in range(2):
                        cs = slice(cc * 256, (cc + 1) * 256)
                        for ot in range(NT):
                            nc.tensor.matmul(
                                ps[(ot, 0)][:, cs],
                                cp12[0][:, :, ot * 128:(ot + 1) * 128],
                                pows[(0, 'dr', 0)][cc][:, :, :],
                                start=False,
                                stop=False,
                                perf_mode=DR,
                            )
                    for ot in range(NT):
                        nc.tensor.matmul(
                            ps[(ot, 1)],
                            cp12[0][:, :, ot * 128:(ot + 1) * 128],
                            pows[(0, 'dr', 1)][:, :, :],
                            start=False,
                            stop=False,
                            perf_mode=DR,
                        )
                else:
                    for ot in range(NT):
                        for h in range(NH):
                            nc.tensor.matmul(
                                ps[(ot, h)],
                                cp12[k][:, :, ot * 128:(ot + 1) * 128],
                                pows[(k, 'dr', h)][:, :, :],
                                start=False,
                                stop=False,
                                perf_mode=DR,
                            )
                    for p in (3, 4):
                        for ot in range(NT):
                            for h in range(NH):
                                nc.tensor.matmul(
                                    ps[(ot, h)],
                                    cp34[(k, p)][:, ot * 128:(ot + 1) * 128],
                                    pows[(k, p, h)][:],
                                    start=False,
                                    stop=False,
                                )

        # trailing k-planes group-contiguous: groups finish staggered, so
        # descale+bias evacuation and output DMA overlap the matmul stream
        inv = 1.0 / CSCALE
        gidx = -1
        for ot in range(NT):
            for h in range(NH):
                gidx += 1
                for k in range(NK - NTAIL, NK):
                    nc.tensor.matmul(
                        ps[(ot, h)],
                        cp12[k][:, :, ot * 128:(ot + 1) * 128],
                        pows[(k, 'dr', h)][:, :, :],
                        start=False,
                        stop=False,
                        perf_mode=DR,
                    )
                    for p in (3, 4):
                        nc.tensor.matmul(
                            ps[(ot, h)],
                            cp34[(k, p)][:, ot * 128:(ot + 1) * 128],
                            pows[(k, p, h)][:],
                            start=False,
                            stop=(k == NK - 1 and p == 4),
                        )
                # descale + bias-add: ONE engine per group (scalar+vector
                # cannot read the same PSUM bank in parallel, so a split
                # evac just serializes); alternate engines across groups so
                # consecutive groups' evacs overlap.  The last group uses
                # the scalar engine (vector has the longer queue by then).
                o_sb = opool.tile([128, 512], F32, tag=f"o_{ot}_{h}",
                                  name=f"o_{ot}_{h}")
                if gidx % 2 == 0:
                    nc.vector.tensor_scalar(
                        o_sb[:],
                        ps[(ot, h)][:],
                        inv,
                        biascol[:, ot:ot + 1],
                        mybir.AluOpType.mult,
                        mybir.AluOpType.add,
                    )
                else:
                    nc.scalar.activation(
                        o_sb[:],
                        ps[(ot, h)][:],
                        mybir.ActivationFunctionType.Identity,
                        bias=biascol[:, ot:ot + 1],
                        scale=inv,
                    )
                # first 6 groups store via the scalar HWDGE ring; the last
                # two use the (by now idle) sync ring, and the final group
                # splits its store across BOTH rings so the two halves
                # drain in parallel right behind the last matmul.
                orow = yt[ot * 128:(ot + 1) * 128, h * 512:(h + 1) * 512]
                if gidx < 6:
                    nc.scalar.dma_start(orow, o_sb[:])
                elif gidx == 6:
                    nc.sync.dma_start(orow, o_sb[:])
                else:
                    nc.sync.dma_start(
                        yt[ot * 128:(ot + 1) * 128, h * 512:h * 512 + 256],
                        o_sb[:, 0:256],
                    )
                    nc.scalar.dma_start(
                        yt[ot * 128:(ot + 1) * 128, h * 512 + 256:(h + 1) * 512],
                        o_sb[:, 256:512],
                    )

    nc.compile()
    return nc


def _get_nc():
    if "nc" not in _CACHE:
        _CACHE["nc"] = _build()
    return _CACHE["nc"]


def _make_in_maps(x, coeffs, bias):
    x = np.asarray(x, dtype=np.float32)
    coeffs = np.asarray(coeffs, dtype=np.float32)
    bias = np.asarray(bias, dtype=np.float32)

    xts = [
        np.ascontiguousarray(x[bg * BS:(bg + 1) * BS, :].T) for bg in range(BW)
    ]
    c12s = [
        np.ascontiguousarray(
            (coeffs[og * OS:(og + 1) * OS, :, 1:3] * CSCALE).transpose(1, 2, 0)
        ).astype(ml_dtypes.float8_e4m3)
        for og in range(OW)
    ]
    c34s = [
        np.ascontiguousarray(
            (coeffs[og * OS:(og + 1) * OS, :, 3:5] * CSCALE).transpose(2, 1, 0)
        ).astype(ml_dtypes.bfloat16)
        for og in range(OW)
    ]
    # biascol[o] = bias[o] + sum_i coeffs[o, i, 0]  (p=0 plane + bias)
    biascol = bias[0] + coeffs[:, :, 0].sum(axis=1)
    in_maps = []
    for c in range(BW * OW):
        bg, og = c // OW, c % OW
        in_maps.append(
            {
                "xt": xts[bg],
                "ct12": c12s[og],
                "ct34": c34s[og],
                "biasc": np.ascontiguousarray(
                    biascol[og * OS:(og + 1) * OS].reshape(OS, 1)
                ).astype(np.float32),
            }
        )
    return in_maps


def _gather(results):
    y = np.empty((B, O), dtype=np.float32)
    for c, res in enumerate(results):
        bg, og = c // OW, c % OW
        y[bg * BS:(bg + 1) * BS, og * OS:(og + 1) * OS] = res["yt"].T
    return y


def run(x, coeffs, bias, trace=False, **trace_kwargs):
    nc = _get_nc()
    in_maps = _make_in_maps(x, coeffs, bias)
    br = run_bass_kernel_spmd(
        nc, in_maps, list(range(BW * OW)), trace=trace, **trace_kwargs
    )
    return _gather(br.results), br


def kernel(x, coeffs, bias):
    out, _ = run(x, coeffs, bias)
    return out
